# revision 1
# baseline (speedup 1.0000x reference)
"""Bahdanau additive attention on 8 TRN2 NeuronCores (Bass/Tile via axon PJRT).

Reference (per batch b):
  Q = hs[b] @ W.T ; K = hs[b] @ U.T                      (S,H)
  scores[q,k] = sum_h v[h] * tanh(Q[q,h] + K[k,h])       (S,S)
  out[b] = softmax(scores, axis=-1) @ hs[b]              (S,H)

Sharding: core c owns batch c//2, query rows [(c%2)*256, +256). Each core
uploads only its 256 query rows of hidden plus a 1/8 shard of W.T/U.T;
on-device AllGathers reconstruct the full hidden[b] (within core pairs)
and the full weights (across all 8) to keep host->device bytes minimal.

Per-core pipeline (H=256 as two 128-partition halves, all fp32):
  - PE transposes hidden -> hidT/hqT; projections on PE give KpT/QpT
    with h_out on partitions.
  - Scores for 16 queries at a time in one fused chain:
    broadcast-AP adds (DVE), one big Tanh (ACT), per-half scale by v
    (ACT, per-partition scale), one gpsimd partition_all_reduce over h,
    then a row-compaction DMA into a (128q, 512k) scores tile.
  - Softmax: exp (scores are bounded by sum|v|, so no max shift), row
    sums + reciprocal; normalization is folded into the output scaling.
  - Context: PE-transpose of the weights, then 4 accumulating matmuls
    against hidden, rows scaled by 1/sum.
"""

import numpy as np

B, S, H = 4, 512, 256
NCORES = 8
QPC = (B * S) // NCORES  # 256 queries per core
HP = 128
CQ = 16                  # queries per fused scores chain
NCH = QPC // CQ          # 16 chunks
KC = S // 128            # 4 key chunks
WSH = H // NCORES        # 32 rows of W.T per core

_CACHE = {}


def _build(reps=1, skip=()):
    import concourse.bass as bass
    import concourse.tile as tile
    import concourse.mybir as mybir
    from concourse import bacc, bass_isa
    from concourse.masks import make_identity
    from contextlib import ExitStack

    f32 = mybir.dt.float32
    AF = mybir.ActivationFunctionType
    ADD = mybir.AluOpType.add

    nc = bacc.Bacc("TRN2", target_bir_lowering=False, debug=False)

    hidhalf = nc.declare_dram_parameter("hidhalf", [QPC, H], f32, isOutput=False)
    WTsh = nc.declare_dram_parameter("WTsh", [WSH, H], f32, isOutput=False)
    UTsh = nc.declare_dram_parameter("UTsh", [WSH, H], f32, isOutput=False)
    vpack = nc.declare_dram_parameter("vpack", [HP, 2], f32, isOutput=False)
    out = nc.declare_dram_parameter("out", [QPC, H], f32, isOutput=True)

    hh_b = nc.dram_tensor("hh_b", [QPC, H], f32)
    wt_b = nc.dram_tensor("wt_b", [WSH, H], f32)
    ut_b = nc.dram_tensor("ut_b", [WSH, H], f32)
    hid_full = nc.dram_tensor("hid_full", [S, H], f32)
    WT_full = nc.dram_tensor("WT_full", [H, H], f32, addr_space="Shared")
    UT_full = nc.dram_tensor("UT_full", [H, H], f32, addr_space="Shared")

    with tile.TileContext(nc) as tc, ExitStack() as ctx:
        sg = ctx.enter_context(tc.tile_pool(name="sg", bufs=1))
        big = ctx.enter_context(tc.tile_pool(name="big", bufs=2))
        scp = ctx.enter_context(tc.tile_pool(name="scp", bufs=2))
        auxp = ctx.enter_context(tc.tile_pool(name="auxp", bufs=2))
        outp = ctx.enter_context(tc.tile_pool(name="outp", bufs=2))
        psm = ctx.enter_context(tc.tile_pool(name="psm", bufs=2, space="PSUM"))

        # own query rows + v (plain loads)
        hq = []
        for t2 in range(2):
            t = sg.tile([HP, H], f32, tag=f"hq{t2}")
            nc.sync.dma_start(out=t, in_=hidhalf[t2 * HP : (t2 + 1) * HP, :])
            hq.append(t)
        sv = sg.tile([HP, 2], f32, tag="v")
        nc.sync.dma_start(out=sv, in_=vpack[:])

        # gather full hidden (pairs) + full weights (all cores)
        sb_hid = [sg.tile([HP, H], f32, tag=f"hid{k}", name=f"hid{k}") for k in range(KC)]
        sb_WT = [sg.tile([HP, H], f32, tag=f"WT{i}", name=f"WT{i}") for i in range(2)]
        sb_UT = [sg.tile([HP, H], f32, tag=f"UT{i}", name=f"UT{i}") for i in range(2)]
        with tc.tile_critical():
            with (
                nc.semaphore("dmasem") as dmasem,
                nc.semaphore("ccsem") as ccsem,
            ):
                nc.gpsimd.dma_start(out=hh_b[:], in_=hidhalf[:]).then_inc(dmasem, 16)
                nc.gpsimd.dma_start(out=wt_b[:], in_=WTsh[:]).then_inc(dmasem, 16)
                nc.gpsimd.dma_start(out=ut_b[:], in_=UTsh[:]).then_inc(dmasem, 16)
                nc.gpsimd.wait_ge(dmasem, 48)
                nc.gpsimd.collective_compute(
                    "AllGather", mybir.AluOpType.bypass,
                    replica_groups=[[0, 1], [2, 3], [4, 5], [6, 7]],
                    ins=[hh_b[:]], outs=[hid_full[:]],
                ).then_inc(ccsem, 1)
                nc.gpsimd.collective_compute(
                    "AllGather", mybir.AluOpType.bypass,
                    replica_groups=[list(range(NCORES))],
                    ins=[wt_b[:]], outs=[WT_full[:]],
                ).then_inc(ccsem, 1)
                nc.gpsimd.collective_compute(
                    "AllGather", mybir.AluOpType.bypass,
                    replica_groups=[list(range(NCORES))],
                    ins=[ut_b[:]], outs=[UT_full[:]],
                ).then_inc(ccsem, 1)
                nc.gpsimd.wait_ge(ccsem, 3)
                for k in range(KC):
                    nc.gpsimd.dma_start(
                        out=sb_hid[k], in_=hid_full[k * HP : (k + 1) * HP, :]
                    ).then_inc(dmasem, 16)
                for i in range(2):
                    nc.gpsimd.dma_start(
                        out=sb_WT[i], in_=WT_full[i * HP : (i + 1) * HP, :]
                    ).then_inc(dmasem, 16)
                    nc.gpsimd.dma_start(
                        out=sb_UT[i], in_=UT_full[i * HP : (i + 1) * HP, :]
                    ).then_inc(dmasem, 16)
                nc.gpsimd.wait_ge(dmasem, 176)

        ident = sg.tile([HP, HP], f32, tag="ident")
        make_identity(nc, ident)

        for rep in range(reps):
            # hidT (h on partitions, all 512 tokens) and hqT (own 256 queries)
            sb_hidT = []
            sb_hqT = []
            for hc in range(2):
                ps = psm.tile([HP, S], f32, tag="ps")
                for k in range(KC):
                    nc.tensor.transpose(
                        ps[:, k * HP : (k + 1) * HP],
                        sb_hid[k][:, hc * HP : (hc + 1) * HP], ident)
                t = auxp.tile([HP, S], f32, tag=f"hidT{hc}")
                nc.vector.tensor_copy(t, ps)
                sb_hidT.append(t)
            for hc in range(2):
                ps = psm.tile([HP, H], f32, tag="ps")
                for t2 in range(2):
                    nc.tensor.transpose(
                        ps[:, t2 * HP : (t2 + 1) * HP],
                        hq[t2][:, hc * HP : (hc + 1) * HP], ident)
                t = auxp.tile([HP, H], f32, tag=f"hqT{hc}")
                nc.vector.tensor_copy(t, ps)
                sb_hqT.append(t)

            # projections: KpT[o,k] / QpT[o,q] with h_out on partitions,
            # both halves packed into one tile along a free axis
            Kp2 = auxp.tile([HP, 2, S], f32, tag="Kp2")
            Qp2 = auxp.tile([HP, 2, H], f32, tag="Qp2")
            for oc in range(2):
                ps = psm.tile([HP, S], f32, tag="ps")
                for hc in range(2):
                    nc.tensor.matmul(
                        ps, lhsT=sb_UT[hc][:, oc * HP : (oc + 1) * HP],
                        rhs=sb_hidT[hc], start=(hc == 0), stop=(hc == 1))
                nc.vector.tensor_copy(Kp2[:, oc], ps)
            for oc in range(2):
                ps = psm.tile([HP, H], f32, tag="ps")
                for hc in range(2):
                    nc.tensor.matmul(
                        ps, lhsT=sb_WT[hc][:, oc * HP : (oc + 1) * HP],
                        rhs=sb_hqT[hc], start=(hc == 0), stop=(hc == 1))
                nc.vector.tensor_copy(Qp2[:, oc], ps)

            for qt in range(QPC // 128):
                s01 = scp.tile([128, 2, S], f32, tag="s01")
                for cc in range(128 // CQ):
                    q0 = qt * 128 + cc * CQ
                    # A[p, half, q, k] = Kp2[p, half, k] + Qp2[p, half, q0+q]
                    A = big.tile([HP, 2, CQ, S], f32, tag="big")
                    k_b = bass.AP(
                        tensor=Kp2.tensor, offset=Kp2.offset,
                        ap=[Kp2.ap[0], [S, 2], [0, CQ], [1, S]])
                    q_b = bass.AP(
                        tensor=Qp2.tensor, offset=Qp2.offset + q0,
                        ap=[Qp2.ap[0], [H, 2], [1, CQ], [0, S]])
                    if "add" not in skip:
                        nc.vector.tensor_tensor(out=A, in0=k_b, in1=q_b, op=ADD)
                    Bt = big.tile([HP, 2, CQ, S], f32, tag="big")
                    if "tanh" not in skip:
                        nc.scalar.activation(Bt, A, AF.Tanh)
                    # C[p, half, q, k] = B * v[p, half]  (broadcast multiply)
                    Ct = big.tile([HP, 2, CQ, S], f32, tag="big")
                    v_b = bass.AP(
                        tensor=sv.tensor, offset=sv.offset,
                        ap=[sv.ap[0], [1, 2], [0, CQ], [0, S]])
                    if "vmul" not in skip:
                        nc.vector.tensor_tensor(out=Ct, in0=Bt, in1=v_b,
                                                op=mybir.AluOpType.mult)
                    R = big.tile([HP, 2, CQ, S], f32, tag="big")
                    if "red" in skip:
                        R = Ct
                    else:
                        nc.gpsimd.partition_all_reduce(
                        R.rearrange("p a b c -> p (a b c)"),
                        Ct.rearrange("p a b c -> p (a b c)"),
                        channels=HP, reduce_op=bass_isa.ReduceOp.add)
                    ro = cc * CQ
                    nc.sync.dma_start(out=s01[ro : ro + CQ, 0], in_=R[0:1, 0])
                    nc.sync.dma_start(out=s01[ro : ro + CQ, 1], in_=R[0:1, 1])

                # softmax pieces (scores bounded by sum|v| ~ 13: exp-safe)
                sc = scp.tile([128, S], f32, tag="sc")
                nc.vector.tensor_tensor(out=sc, in0=s01[:, 0], in1=s01[:, 1], op=ADD)
                wts = scp.tile([128, S], f32, tag="wts")
                nc.scalar.activation(wts, sc, AF.Exp)
                ssum = auxp.tile([128, 1], f32, tag="ssum")
                nc.vector.tensor_reduce(
                    ssum, wts, axis=mybir.AxisListType.X, op=mybir.AluOpType.add)
                rinv = auxp.tile([128, 1], f32, tag="rinv")
                nc.vector.reciprocal(rinv, ssum)

                # context: wtsT chunks via PE transpose, then 4 matmuls
                ps_t = psm.tile([HP, S], f32, tag="ps")
                for kc in range(KC):
                    nc.tensor.transpose(
                        ps_t[:, kc * HP : (kc + 1) * HP],
                        wts[:, kc * HP : (kc + 1) * HP], ident)
                wtsT = scp.tile([128, S], f32, tag="wtsT")
                nc.vector.tensor_copy(wtsT, ps_t)
                pctx = psm.tile([128, H], f32, tag="ps")
                for kc in range(KC):
                    nc.tensor.matmul(
                        pctx, lhsT=wtsT[:, kc * HP : (kc + 1) * HP],
                        rhs=sb_hid[kc], start=(kc == 0), stop=(kc == KC - 1))
                octx = outp.tile([128, H], f32, tag="octx")
                nc.vector.tensor_scalar_mul(octx, pctx, rinv)
                nc.sync.dma_start(out=out[qt * 128 : (qt + 1) * 128, :], in_=octx)

    nc.compile()
    return nc


def _get(reps=1, skip=()):
    key = (reps, tuple(skip))
    if key not in _CACHE:
        _CACHE[key] = _build(reps, skip)
    return _CACHE[key]


def _in_maps(hs, W, U, v):
    hs = np.asarray(hs, np.float32)
    WTh = np.ascontiguousarray(np.asarray(W, np.float32).T)
    UTh = np.ascontiguousarray(np.asarray(U, np.float32).T)
    vp = np.ascontiguousarray(np.asarray(v, np.float32).reshape(2, HP).T)
    maps = []
    for c in range(NCORES):
        b, qh = divmod(c, 2)
        maps.append({
            "hidhalf": np.ascontiguousarray(hs[b, qh * QPC : (qh + 1) * QPC]),
            "WTsh": np.ascontiguousarray(WTh[c * WSH : (c + 1) * WSH]),
            "UTsh": np.ascontiguousarray(UTh[c * WSH : (c + 1) * WSH]),
            "vpack": vp,
        })
    return maps


def run(hidden_states, W, U, v, reps=1, skip=()):
    from concourse.bass_utils import run_bass_kernel_spmd

    nc = _get(reps, skip)
    res = run_bass_kernel_spmd(
        nc, _in_maps(hidden_states, W, U, v), core_ids=list(range(NCORES)))
    ctxout = np.empty((B, S, H), np.float32)
    for c in range(NCORES):
        b, qh = divmod(c, 2)
        ctxout[b, qh * QPC : (qh + 1) * QPC] = res.results[c]["out"]
    return ctxout


def kernel(**inputs):
    return run(inputs["hidden_states"], inputs["W"], inputs["U"], inputs["v"])

